# revision 35
# baseline (speedup 1.0000x reference)
"""KMeans dataset imputation on 8 Trainium2 NeuronCores.

Math: for each batch row b (masked squared distance to 512 centers):
    d[b,k] = sum_j m[b,j]*(x[b,j]-c[k,j])^2 = xx[b] - 2*xc[b,k] + cc[b,k]
argmin_k d is independent of xx[b], so we compute
    s[b,k] = 2*xc - cc = (m*x) @ w1 + m @ w2,   w1 = 2c^T, w2 = -(c^2)^T
and take argmax_k s. Output row = bank[argmax], bank = data_to_impute[per_cluster_index].

Precision: split s into a bf16 "main" part and an fp8 "correction" part:
    main = mxh @ w1h + mb @ w2h                       (bf16 matmuls)
    corr = mxh @ w1l + mxl @ w1h + mb @ w2l           (fp8e4 DoubleRow matmuls)
where xh/xl, w1h/w1l, w2h/w2l are bf16 hi/lo splits, m is {0,1} (exact),
mxh = m*xh (exact in bf16). The low-order terms are scaled by 2^8 so their
fp8 encodings stay in E4M3 normal range; the main terms' weights carry the
same 2^8 so the two PSUM accumulators can be combined with a plain add
(argmax is scale-invariant). Validated on the real inputs: 0 argmin flips
vs the fp64 reference.

Layout: contraction streams are concatenated row-wise, zero-padded to a
multiple of 128, and chunked as dram[p, unit, col] = rows[unit*128+p, col].
bf16 units are consumed one chunk per matmul; fp8 units two chunks per
DoubleRow matmul (pair dim = unit axis slice of size 2). A DoubleRow MM
streams its 512 columns at the same rate as a bf16 MM, so the three
correction passes cost 10 MMs/btile instead of 24.

The output bank rows are shipped and returned in bf16 (rel err ~0.0017,
gate is 2e-2), halving the gather + store traffic; the host converts the
gathered rows back to f32.

Schedule per core (1024 batch rows = 8 btiles, two quads of 4):
  quad 0 unit-outer: 10 DoubleRow MMs x4 btiles -> corr PSUMs, DVE copies
  free the banks, then 13 bf16 MMs x4 -> main PSUMs, then 4 epilogues.
  quad 1 btile-outer (weights resident by then): per btile corr MMs,
  copy, main MMs, epilogue - btiles finish ~2.8us apart so the epilogues
  overlap the MM stream and only the last btile's epilogue is exposed.
  Epilogue: DVE add (main+corr PSUM), MAX8, max_index; gpsimd indirect
  DMA gathers the bank rows; a direct DMA stores them to the output.
HW-crash learnings (this runtime): tensor_tensor_reduce and any
indirect-DMA whose offset AP starts at a nonzero partition kill the
device; both are avoided (USE_TTR / USE_HALVES stay False).
"""

from contextlib import ExitStack

import ml_dtypes
import numpy as np

import concourse.bass as bass
import concourse.tile as tile
from concourse import bacc, mybir
from concourse.bass_utils import run_bass_kernel_spmd

N_CORES = 8
B, D, K = 8192, 784, 512
BL = B // N_CORES          # 1024 batch rows per core
P = 128
NU_M = 13                  # main bf16 row-chunks: 2*784=1568 -> 13*128
NU_C = 20                  # corr fp8 row-chunks: 3*784=2352 -> 20*128
NT = NU_C // 2             # 10 DoubleRow units (2 chunks each)
QB = 4                     # btiles per quad
NQ = BL // (QB * P)        # 2 quads

f32 = mybir.dt.float32
bf16 = mybir.dt.bfloat16
fp8 = mybir.dt.float8e4
f8np = ml_dtypes.float8_e4m3
np_bf16 = ml_dtypes.bfloat16

_last_results = None  # test harness reads exec_time_ns from here
USE_DR = True  # DoubleRow fp8 matmuls
USE_SCALAR_DMA = False  # measured slower: A-DMAs on the Activation HWDGE queue
USE_TTR = False  # KEEP False: tensor_tensor_reduce crashes the device
USE_HALVES = False  # KEEP False: offset APs at partition 64 crash the device
USE_VMEMSET = True  # memset warm tile on DVE (fast start) instead of gpsimd


def _build():
    nc = bacc.Bacc("TRN2", debug=False, num_devices=N_CORES)
    a_main = nc.dram_tensor("a_main", [P, NU_M, BL], bf16, kind="ExternalInput").ap()
    w_main = nc.dram_tensor("w_main", [P, NU_M, K], bf16, kind="ExternalInput").ap()
    a_corr = nc.dram_tensor("a_corr", [P, NU_C, BL], fp8, kind="ExternalInput").ap()
    w_corr = nc.dram_tensor("w_corr", [P, NU_C, K], fp8, kind="ExternalInput").ap()
    bank = nc.dram_tensor("bank", [K, D], bf16, kind="ExternalInput").ap()
    out = nc.dram_tensor("out", [BL, D], bf16, kind="ExternalOutput").ap()

    DR = mybir.MatmulPerfMode.DoubleRow
    ADD = mybir.AluOpType.add
    MAX = mybir.AluOpType.max
    QW = QB * P            # 512 batch cols per quad

    with tile.TileContext(nc) as tc, ExitStack() as ctx:
        io = ctx.enter_context(tc.tile_pool(name="io", bufs=1))
        epi = ctx.enter_context(tc.tile_pool(name="epi", bufs=4))
        scp = ctx.enter_context(tc.tile_pool(name="scp", bufs=1))
        mxp = ctx.enter_context(tc.tile_pool(name="mxp", bufs=1))
        psp = ctx.enter_context(tc.tile_pool(name="psp", bufs=8, space="PSUM"))

        # Dep-free warm-up matmuls: keep the PE busy during the initial DMA
        # wait so HAM un-throttles before the real matmuls start. Small N so
        # the first real matmul isn't stuck behind a long warm-up.
        warm = io.tile([P, P], bf16, tag="warm")
        (nc.vector if USE_VMEMSET else nc.gpsimd).memset(warm[:], 0)
        wps = psp.tile([P, K], f32, tag="ps", name="wps")
        for _ in range(30):
            nc.tensor.matmul(wps[:, :P], warm[:], warm[:], start=True, stop=True)

        # max-value tiles: ttr's accum writes lane 0; lanes 1-7 stay 0 (the
        # scores are all negative, so lane 0 holds the strict max and the
        # unused lanes never match anything in max_index).
        mx_t = []
        if USE_TTR:
            for i in range(8):
                mx = mxp.tile([P, 8], f32, tag=f"mx{i}", name=f"mx{i}")
                (nc.vector if USE_VMEMSET else nc.gpsimd).memset(mx[:], 0)
                mx_t.append(mx)

        # Input DMAs, issued up front in consumption order: fine-grained at
        # the head (first matmul starts sooner), coarse later (fewer issue
        # slots, 4KB contiguous runs). AP lookup tables map (t|u, q) to the
        # right tile slice.
        ac_ap = [[None] * NQ for _ in range(NT)]   # [t][q] -> lhsT AP [P,2,QW]
        wc_ap = [None] * NT                        # [t] -> rhs AP [P,2,K]
        am_ap = [[None] * NQ for _ in range(NU_M)] # [u][q] -> lhsT AP [P,1,QW]
        wm_ap = [None] * NU_M                      # [u] -> rhs AP [P,1,K]

        def load_ac(ts_, te, q0only=False):
            nu = 2 * (te - ts_)
            qr = 1 if q0only else NQ
            cols = qr * QW
            tl = io.tile([P, nu, cols], fp8, tag=f"ac{ts_}_{q0only}", name=f"ac{ts_}_{q0only}")
            nc.sync.dma_start(tl[:], a_corr[:, 2 * ts_ : 2 * ts_ + nu, 0:cols])
            for t in range(ts_, te):
                for q in range(qr):
                    ac_ap[t][q] = tl[:, 2 * (t - ts_) : 2 * (t - ts_) + 2, q * QW : (q + 1) * QW]

        def load_ac_q1(ts_, te):
            nu = 2 * (te - ts_)
            tl = io.tile([P, nu, QW], fp8, tag=f"acq1_{ts_}", name=f"acq1_{ts_}")
            nc.sync.dma_start(tl[:], a_corr[:, 2 * ts_ : 2 * ts_ + nu, QW : 2 * QW])
            for t in range(ts_, te):
                ac_ap[t][1] = tl[:, 2 * (t - ts_) : 2 * (t - ts_) + 2, :]

        def load_wc(ts_, te):
            nu = 2 * (te - ts_)
            tl = io.tile([P, nu, K], fp8, tag=f"wc{ts_}", name=f"wc{ts_}")
            nc.sync.dma_start(tl[:], w_corr[:, 2 * ts_ : 2 * ts_ + nu, :])
            for t in range(ts_, te):
                wc_ap[t] = tl[:, 2 * (t - ts_) : 2 * (t - ts_) + 2, :]

        def load_am(us, ue):
            nu = ue - us
            tl = io.tile([P, nu, BL], bf16, tag=f"am{us}", name=f"am{us}")
            nc.sync.dma_start(tl[:], a_main[:, us:ue, :])
            for u in range(us, ue):
                for q in range(NQ):
                    am_ap[u][q] = tl[:, u - us : u - us + 1, q * QW : (q + 1) * QW]

        def load_wm(us, ue):
            nu = ue - us
            tl = io.tile([P, nu, K], bf16, tag=f"wm{us}", name=f"wm{us}")
            nc.sync.dma_start(tl[:], w_main[:, us:ue, :])
            for u in range(us, ue):
                wm_ap[u] = tl[:, u - us : u - us + 1, :]

        for t in range(NT):
            load_ac(t, t + 1)
            if t % 2 == 0:
                load_wc(t, t + 2)
        for u in range(NU_M):
            load_am(u, u + 1)
            if u % 2 == 0:
                load_wm(u, min(u + 2, NU_M))

        def corr_mms(q, b, ps):
            for t in range(NT):
                wc = wc_ap[t]
                lhs = ac_ap[t][q]
                if USE_DR:
                    nc.tensor.matmul(
                        ps[:], lhs[:, :, bass.ts(b, P)], wc,
                        start=(t == 0), stop=(t == NT - 1), perf_mode=DR,
                    )
                else:
                    for j in range(2):
                        nc.tensor.matmul(
                            ps[:], lhs[:, j : j + 1, bass.ts(b, P)], wc[:, j : j + 1, :],
                            start=(t == 0 and j == 0), stop=(t == NT - 1 and j == 1),
                        )

        def main_mms(q, b, ps):
            for u in range(NU_M):
                wm = wm_ap[u]
                lhs = am_ap[u][q]
                nc.tensor.matmul(
                    ps[:], lhs[:, :, bass.ts(b, P)], wm,
                    start=(u == 0), stop=(u == NU_M - 1),
                )

        def epilogue(q, b, ps_m_b, sc_c_b):
            g_b = q * QB + b
            sc = epi.tile([P, K], f32, tag="sc", name=f"sc{g_b}")
            if USE_TTR:
                nc.vector.tensor_tensor_reduce(
                    out=sc[:],
                    in0=ps_m_b[:],
                    in1=sc_c_b[:],
                    scale=1.0,
                    scalar=-3.0e38,
                    op0=ADD,
                    op1=MAX,
                    accum_out=mx_t[g_b][:, :1],
                )
                mxv8 = mx_t[g_b][:]
            else:
                nc.vector.tensor_add(sc[:], ps_m_b[:], sc_c_b[:])
                mxv = epi.tile([P, 8], f32, tag="mxv", name=f"mxv{g_b}")
                nc.vector.max(mxv[:], sc[:])
                mxv8 = mxv[:]
            idx8 = epi.tile([P, 8], mybir.dt.uint32, tag="idx8", name=f"idx8{g_b}")
            nc.vector.max_index(idx8[:], mxv8, sc[:])
            if USE_HALVES:
                H = P // 2
                for h in range(2):
                    gh = epi.tile([H, D], bf16, tag=f"gh{h}", name=f"gh{h}_{g_b}")
                    nc.gpsimd.indirect_dma_start(
                        out=gh[:],
                        out_offset=None,
                        in_=bank[:],
                        in_offset=bass.IndirectOffsetOnAxis(ap=idx8[bass.ts(h, H), :1], axis=0),
                    )
                    nc.sync.dma_start(out[g_b * P + h * H : g_b * P + (h + 1) * H, :], gh[:])
            else:
                g = epi.tile([P, D], bf16, tag="g", name=f"g{g_b}")
                nc.gpsimd.indirect_dma_start(
                    out=g[:],
                    out_offset=None,
                    in_=bank[:],
                    in_offset=bass.IndirectOffsetOnAxis(ap=idx8[:, :1], axis=0),
                )
                if g_b == 7:
                    # split the exposed last store by columns across both
                    # HWDGE queues so the two halves transfer in parallel
                    h = D // 2
                    nc.sync.dma_start(out[g_b * P : (g_b + 1) * P, :h], g[:, :h])
                    nc.scalar.dma_start(out[g_b * P : (g_b + 1) * P, h:], g[:, h:])
                else:
                    nc.sync.dma_start(out[g_b * P : (g_b + 1) * P, :], g[:])

        # Quad 0: unit-outer so the W stream is touched at 1/4 rate while
        # it loads, and the scheduler can interleave btiles as units land.
        # Quad 1: btile-outer (everything resident by then) so btiles
        # finish staggered and their epilogues overlap the MM stream.
        ps_c = [psp.tile([P, K], f32, tag="ps", name=f"psc0_{b}") for b in range(QB)]
        ps_m = [psp.tile([P, K], f32, tag="ps", name=f"psm0_{b}") for b in range(QB)]

        for t in range(NT):
            wc = wc_ap[t]
            lhs = ac_ap[t][0]
            for b in range(QB):
                if USE_DR:
                    nc.tensor.matmul(
                        ps_c[b][:], lhs[:, :, bass.ts(b, P)], wc,
                        start=(t == 0), stop=(t == NT - 1), perf_mode=DR,
                    )
                else:
                    for j in range(2):
                        nc.tensor.matmul(
                            ps_c[b][:], lhs[:, j : j + 1, bass.ts(b, P)], wc[:, j : j + 1, :],
                            start=(t == 0 and j == 0), stop=(t == NT - 1 and j == 1),
                        )

        # free the corr PSUM banks early (ACT engine: otherwise idle, and
        # keeping these off the DVE queue keeps the epilogue chains tight)
        sc_c = []
        for b in range(QB):
            s = scp.tile([P, K], f32, tag=f"scc0{b}", name=f"scc0{b}")
            nc.scalar.copy(s[:], ps_c[b][:])
            sc_c.append(s)

        for u in range(NU_M):
            wm = wm_ap[u]
            lhs = am_ap[u][0]
            for b in range(QB):
                nc.tensor.matmul(
                    ps_m[b][:], lhs[:, :, bass.ts(b, P)], wm,
                    start=(u == 0), stop=(u == NU_M - 1),
                )

        for b in range(QB):
            epilogue(0, b, ps_m[b], sc_c[b])

        for b in range(QB):
            psc = psp.tile([P, K], f32, tag="ps", name=f"psc1_{b}")
            corr_mms(1, b, psc)
            scb = scp.tile([P, K], f32, tag=f"scc1{b}", name=f"scc1{b}")
            nc.scalar.copy(scb[:], psc[:])
            psm = psp.tile([P, K], f32, tag="ps", name=f"psm1_{b}")
            main_mms(1, b, psm)
            epilogue(1, b, psm, scb)

    nc.compile()
    return nc


def _pack(blocks, nu, ncols, dt):
    """Stack row blocks, zero-pad to nu*128 rows, chunk to [128, nu, ncols]."""
    rows = np.concatenate(blocks, axis=0)
    padded = np.zeros((nu * P, ncols), dtype=dt)
    padded[: rows.shape[0]] = rows
    return np.ascontiguousarray(padded.reshape(nu, P, ncols).transpose(1, 0, 2))


def kernel(data, mask, centers, data_to_impute, per_cluster_index):
    global _last_results
    x = np.asarray(data, dtype=np.float32).reshape(B, D)
    m = np.asarray(mask, dtype=np.float32).reshape(B, D)
    c = np.asarray(centers, dtype=np.float32)

    xh32 = x.astype(np_bf16).astype(np.float32)
    mxh = (m * xh32).astype(np_bf16)          # exact: m is {0,1}
    mxl = m * (x - xh32)
    mb = m.astype(np_bf16)

    w1 = np.ascontiguousarray((2.0 * c).T).astype(np.float32)      # [D, K]
    w1h32 = w1.astype(np_bf16).astype(np.float32)
    w2 = np.ascontiguousarray((-(c * c)).T).astype(np.float32)
    w2h32 = w2.astype(np_bf16).astype(np.float32)

    a_main = _pack([mxh.T, mb.T], NU_M, B, np_bf16)
    w_main = _pack(
        [(w1h32 * 256.0).astype(np_bf16), (w2h32 * 256.0).astype(np_bf16)],
        NU_M, K, np_bf16,
    )
    a_corr = _pack(
        [mxh.astype(f8np).T, (mxl * 256.0).astype(f8np).T, mb.astype(f8np).T],
        NU_C, B, f8np,
    )
    w_corr = _pack(
        [
            ((w1 - w1h32) * 256.0).astype(f8np),
            w1h32.astype(f8np),
            ((w2 - w2h32) * 256.0).astype(f8np),
        ],
        NU_C, K, f8np,
    )

    pci = np.asarray(per_cluster_index).astype(np.int64)
    bank_h = np.ascontiguousarray(
        np.asarray(data_to_impute, dtype=np.float32)[pci].astype(np_bf16))

    in_maps = []
    for core in range(N_CORES):
        sl = slice(core * BL, (core + 1) * BL)
        in_maps.append(
            {
                "a_main": np.ascontiguousarray(a_main[:, :, sl]),
                "w_main": w_main,
                "a_corr": np.ascontiguousarray(a_corr[:, :, sl]),
                "w_corr": w_corr,
                "bank": bank_h,
            }
        )

    nc = _build()
    res = run_bass_kernel_spmd(nc, in_maps, core_ids=list(range(N_CORES)))
    _last_results = res
    out = np.concatenate([res.results[cc]["out"] for cc in range(N_CORES)], axis=0)
    return out.reshape(np.asarray(data).shape).astype(np.float32)
